# revision 22
# baseline (speedup 1.0000x reference)
"""Trainium2 Bass kernel for the MultiHeadAttention problem (B=4,S=2048,D=1024,H=16).

Math implemented (matches reference.py, including its quirks):
  x = q  (k, v inputs ignored by the reference)
  Qh/Kh/Vh from x*proj_{Q,K,V}, head h uses channels [h*64,(h+1)*64)
  scores = Qh @ Kh^T / sqrt(D); mask index for (b,h) is (b*H+h) % B
  masked scores -> -1e-10, so exp(masked) == 1.0f exactly in fp32
  softmax (no max-subtraction needed: |scores| small), ctx = attn @ Vh
  out = LayerNorm(ctx + q) * gamma + beta

Device decomposition per core (8 cores; core c -> batch b=c//2, query half c%2):
  scoresT[k,q] = (qT*w).T-chunk @ qT  with w = projQ*projK/sqrt(D)  (bf16 mm)
  A = (exp(scoresT) - 1) * notm      (ACT exp FD=2048; DVE ts_sub; DVE TT 2x)
  ctxT[dd+Z, q] = sum_kc xV_h.T @ A  + colsum(xV_h) x ones   (stationary xV,
     A streams 512-col moving operands -> high PE duty keeps HAM un-throttled)
  ctx = transpose(ctxT) per q-block via PE; normalize by Z col; + residual; LN.
"""

import numpy as np
import ml_dtypes

bf16 = ml_dtypes.bfloat16
f8e4 = ml_dtypes.float8_e4m3
B, S, D, H, DH = 4, 2048, 1024, 16, 64
HALF = S // 2  # 1024 query rows per core
NCORES = 8
LN_EPS = 1e-5

_CACHE = {}


def _patch_tile_drain(tile, mybir, bass_rust):
    """The walrus build in this env allows only one sem wait per (non-event)
    instruction; TileContext's exit drain can collect several (one per
    outstanding DMA queue).  Spread them over a chain of single-wait drains."""
    if getattr(tile.TileContext, "_drain_patched", False):
        return

    def _patched(self, tick_clock, wait_clock):
        drain_inst = self.nc.sync.drain()
        wait_clock.add_sem_waits(
            drain_inst.ins, bass_rust.ScopedClock({None: tick_clock.global_clock})
        )
        ii = drain_inst.ins
        waits = list(ii.sync_info.on_wait) if ii.sync_info else []
        if len(waits) > 1:
            ii.sync_info = mybir.SyncInfo(on_wait=[waits[0]], on_update=[])
            for w in waits[1:]:
                extra = self.nc.sync.drain()
                extra.ins.sync_info = mybir.SyncInfo(on_wait=[w], on_update=[])
        self.nc.all_engine_barrier()
        popped = self.nc._tile_sem_poison_stack.pop()
        assert popped is self._sem_poison
        self.nc.clear_and_free_semaphores(list(self.sems.allocated().values()))
        self.nc.all_engine_barrier()

    tile.TileContext._drain_and_barrier = _patched
    tile.TileContext._drain_patched = True


def _split_multi_waits(nc, mybir):
    """This env's walrus accepts only one sync wait per instruction (two for
    EventSemaphore).  Hoist extra waits onto preceding same-engine NoOps —
    engines are in-order, so semantics are identical."""
    for f in nc.m.functions:
        for blk in f.blocks:
            out = []
            changed = False
            for inst in blk.instructions:
                si = inst.sync_info
                waits = list(si.on_wait) if si and si.on_wait else []
                limit = 2 if isinstance(inst, mybir.InstEventSemaphore) else 1
                if len(waits) > limit:
                    changed = True
                    for i, w in enumerate(waits[: len(waits) - limit]):
                        nop = mybir.InstNoOp(name=f"{inst.name}.w{i}", ins=[], outs=[])
                        nop.engine = inst.engine
                        nop.sync_info = mybir.SyncInfo(on_wait=[w], on_update=[])
                        out.append(nop)
                    inst.sync_info = mybir.SyncInfo(
                        on_wait=waits[len(waits) - limit :],
                        on_update=list(si.on_update) if si.on_update else [],
                    )
                out.append(inst)
            if changed:
                blk.instructions = out


def _build_nc():
    import concourse.bass as bass
    import concourse.mybir as mybir
    import concourse.tile as tile
    import bass_rust

    _patch_tile_drain(tile, mybir, bass_rust)

    f32 = mybir.dt.float32
    b16 = mybir.dt.bfloat16
    i32 = mybir.dt.int32
    EXP = mybir.ActivationFunctionType.Exp
    MULT = mybir.AluOpType.mult
    ADD = mybir.AluOpType.add
    SUB = mybir.AluOpType.subtract
    SHR = mybir.AluOpType.arith_shift_right

    f8 = mybir.dt.float8e4
    DR = mybir.MatmulPerfMode.DoubleRow

    nc = bass.Bass(dynamic_dma_scratch_size=2048)

    # fp8 DoubleRow layout for mm1: channel c of head h lives at
    # [p, h, i, :] with c = h*64 + i*32 + p  (contraction = 2 subtiles x 32)
    # head h lives in partitions [32(h%3), 32(h%3)+32), slot h//3 (SBUF AP
    # base partition must be 0/32/64); channel c = h*64 + i*32 + p_local
    # (DoubleRow: 2 subtiles x 32 rows)
    qTw = nc.dram_tensor("qTw", [128, 6, 2, S], f8, kind="ExternalInput")
    qTr = nc.dram_tensor("qTr", [128, 6, 2, HALF], f8, kind="ExternalInput")
    xV = nc.dram_tensor("xV", [S, H * 65], b16, kind="ExternalInput")
    # kc-PAIR layout: notmT[g][pc][p][0:1024]   = 1-mask[g][q, k=256*pc+p]
    #                 notmT[g][pc][p][1024:2048]= 1-mask[g][q, k=256*pc+128+p]
    notmT = nc.dram_tensor("notmT", [4, 8, 128, 2 * HALF], b16, kind="ExternalInput")
    qres = nc.dram_tensor("qres", [HALF, D], f32, kind="ExternalInput")
    colsum = nc.dram_tensor("colsum", [1, H * 65], b16, kind="ExternalInput")
    ident = nc.dram_tensor("ident", [128, 128], b16, kind="ExternalInput")
    out = nc.dram_tensor("out", [HALF, D], f32, kind="ExternalOutput")

    with tile.TileContext(nc) as tc:
        with (
            tc.tile_pool(name="persist", bufs=1) as P,
            tc.tile_pool(name="notm", bufs=1) as NM,
            tc.tile_pool(name="abuf", bufs=2) as AB,
            tc.tile_pool(name="ctxs", bufs=2) as CS,
            tc.tile_pool(name="small", bufs=4) as SM,
            tc.tile_pool(name="lnbuf", bufs=2) as LB,
            tc.tile_pool(name="ps_s", bufs=1, space="PSUM") as PS,
            tc.tile_pool(name="ps_c", bufs=1, space="PSUM") as PC,
            tc.tile_pool(name="ps_t", bufs=2, space="PSUM") as PT,
        ):
            # ---- persistent loads
            qtw_t = P.tile([128, 6, 2, S], f8, tag="qtwf", name="qtwf")
            nc.sync.dma_start(qtw_t[:], qTw[:, :, :, :])
            qtr_t = P.tile([128, 6, 2, HALF], f8, tag="qtrf", name="qtrf")
            nc.sync.dma_start(qtr_t[:], qTr[:, :, :, :])
            xv = []
            for kc in range(16):
                t = P.tile([128, H * 65], b16, tag=f"xv{kc}", name=f"xv{kc}")
                nc.sync.dma_start(t[:], xV[kc * 128 : (kc + 1) * 128, :])
                xv.append(t)
            cs_t = P.tile([1, H * 65], b16, tag="colsum")
            nc.sync.dma_start(cs_t[:], colsum[:, :])
            id_t = P.tile([128, 128], b16, tag="ident")
            nc.sync.dma_start(id_t[:], ident[:, :])
            ones_row = P.tile([1, HALF], b16, tag="ones_row")
            nc.vector.memset(ones_row[:], 1.0)
            magic_t = P.tile([128, 1], i32, tag="magic")
            nc.vector.memset(magic_t[:], 0x5F3759DF)
            asm = [
                P.tile([128, D], b16, tag=f"asm{qb}", name=f"asm{qb}")
                for qb in range(8)
            ]

            # 2-deep software pipeline over heads:
            #   step h: mm1(h) | mm2(h-1) | transpose+normalize(h-2)
            nm_cur = {}  # pc -> notm tile for current g

            def load_notm(g):
                for pc in range(8):
                    t = NM.tile([128, 2 * HALF], b16, tag=f"nm{pc}", name=f"nm{g}_{pc}")
                    nc.sync.dma_start(t[:], notmT[g, pc, :, :])
                    nm_cur[pc] = t

            heads = [g + 4 * i for g in range(4) for i in range(4)]
            prev1 = None  # (h, atiles, pcT)     head h-1
            prev2 = None  # (h, ctxs)            head h-2
            for idx in range(len(heads) + 2):
                h = heads[idx] if idx < len(heads) else None
                if h is not None:
                    if idx % 4 == 0:
                        load_notm(h % 4)
                    atiles = [
                        AB.tile(
                            [128, 2 * HALF], b16, tag=f"A{pc}", name=f"A{h}_{pc}"
                        )
                        for pc in range(8)
                    ]
                    pcT = PC.tile([65, HALF], f32, tag="ctxT", name=f"ctxT{h}")
                for kc in range(16):
                    pc, hf = kc // 2, kc % 2
                    if h is not None:
                        # mm1: scores^T chunk kc -> psum [128, 1024]
                        # fp8e4m3 DoubleRow: contraction 64 = 2 subtiles x 32
                        ps = PS.tile([128, HALF], f32, tag="scores", name=f"s{h}_{kc}")
                        hg, hs = h % 3, h // 3
                        pb = 32 * hg
                        for qt in range(2):
                            nc.tensor.matmul(
                                ps[:, qt * 512 : (qt + 1) * 512],
                                qtw_t[pb : pb + 32, hs, :, kc * 128 : (kc + 1) * 128],
                                qtr_t[pb : pb + 32, hs, :, qt * 512 : (qt + 1) * 512],
                                start=True,
                                stop=True,
                                perf_mode=DR,
                            )
                    if prev1 is not None:
                        # mm2: accumulate ctxT(h-1) from A(h-1) chunk kc
                        h1, at1, pcT1 = prev1
                        lhs = xv[kc][:, h1 * 65 : (h1 + 1) * 65]
                        for qt in range(2):
                            nc.tensor.matmul(
                                pcT1[:, qt * 512 : (qt + 1) * 512],
                                lhs,
                                at1[pc][:, hf * HALF + qt * 512 : hf * HALF + (qt + 1) * 512],
                                start=(kc == 0),
                                stop=False,
                                skip_group_check=True,
                            )
                    if prev2 is not None and kc < 8:
                        # transpose + normalize head h-2, q-block kc
                        h2, ctxs2 = prev2
                        qb = kc
                        tp = PT.tile([128, 65], b16, tag="tp", name=f"tp{h2}_{qb}")
                        nc.tensor.transpose(
                            tp[:], ctxs2[:, qb * 128 : (qb + 1) * 128], id_t[0:65, 0:65]
                        )
                        rz = SM.tile([128, 1], f32, tag="rz")
                        nc.vector.reciprocal(rz[:], tp[:, 64:65])
                        nc.vector.tensor_scalar_mul(
                            asm[qb][:, h2 * 64 : (h2 + 1) * 64], tp[:, 0:64], rz[:]
                        )
                    if h is not None:
                        at = atiles[pc]
                        nc.scalar.activation(
                            at[:, hf * HALF : (hf + 1) * HALF], ps[:], EXP
                        )
                        if hf == 1:
                            nc.vector.tensor_scalar_add(at[:], at[:], -1.0)
                            nc.vector.tensor_tensor(
                                at[:], at[:], nm_cur[pc][:], op=MULT
                            )
                if prev1 is not None:
                    # rank-1 colsum add + bounce ctxT(h-1) to SBUF
                    h1, at1, pcT1 = prev1
                    for qt in range(2):
                        nc.tensor.matmul(
                            pcT1[:, qt * 512 : (qt + 1) * 512],
                            cs_t[:, h1 * 65 : (h1 + 1) * 65],
                            ones_row[:, qt * 512 : (qt + 1) * 512],
                            start=False,
                            stop=(qt == 1),
                            skip_group_check=True,
                        )
                    ctxs = CS.tile([65, HALF], b16, tag="ctxs", name=f"ctxs{h1}")
                    nc.scalar.copy(ctxs[:], pcT1[:])
                    prev2 = (h1, ctxs)
                prev1 = (h, atiles, pcT) if h is not None else None

            # ---- residual + LayerNorm (rsqrt via Newton on DVE: no ACT
            # table switch away from exp)
            for qb in range(8):
                qr = LB.tile([128, D], f32, tag="qr")
                nc.sync.dma_start(qr[:], qres[qb * 128 : (qb + 1) * 128, :])
                ot = LB.tile([128, D], f32, tag="ot")
                nc.vector.tensor_tensor(ot[:], qr[:], asm[qb][:], op=ADD)
                # mean/var in one DVE pass: bn_stats over two 512 groups
                st = SM.tile([128, 2, 6], f32, tag="st")
                nc.vector.bn_stats(st[:, 0, :], ot[:, 0:512])
                nc.vector.bn_stats(st[:, 1, :], ot[:, 512:1024])
                mv = SM.tile([128, 2], f32, tag="mv")
                nc.vector.bn_aggr(mv[:], st[:])
                mu = SM.tile([128, 1], f32, tag="mu")
                nc.vector.tensor_copy(mu[:], mv[:, 0:1])
                var = SM.tile([128, 1], f32, tag="var")
                nc.vector.tensor_scalar_add(var[:], mv[:, 1:2], LN_EPS)
                # y = rsqrt(var): bit-magic seed + 3 Newton iterations
                t1 = SM.tile([128, 1], i32, tag="t1")
                nc.vector.tensor_scalar(t1[:], var[:].bitcast(i32), 1, None, op0=SHR)
                y = SM.tile([128, 1], f32, tag="y")
                nc.vector.tensor_tensor(y[:].bitcast(i32), magic_t[:], t1[:], op=SUB)
                t2 = SM.tile([128, 1], f32, tag="t2")
                for _ in range(3):
                    nc.vector.tensor_tensor(t2[:], y[:], y[:], op=MULT)
                    nc.vector.tensor_tensor(t2[:], t2[:], var[:], op=MULT)
                    nc.vector.tensor_scalar(t2[:], t2[:], -0.5, 1.5, op0=MULT, op1=ADD)
                    nc.vector.tensor_tensor(y[:], y[:], t2[:], op=MULT)
                nc.vector.tensor_scalar(ot[:], ot[:], mu[:], y[:], op0=SUB, op1=MULT)
                nc.sync.dma_start(out[qb * 128 : (qb + 1) * 128, :], ot[:])

    _split_multi_waits(nc, mybir)
    return nc


def _prep_inputs(q, masks, proj_Q, proj_K, proj_V):
    """Host-side shard prep. Returns list of 8 in_maps."""
    q = np.asarray(q, dtype=np.float32)
    masks = np.asarray(masks)
    w = (proj_Q.astype(np.float64) * proj_K.astype(np.float64) / np.sqrt(D)).astype(
        np.float32
    )

    # notmT[g][k, q] = 1 - masks[g][q, k]  -> [4, S(k), S(q)] bf16
    notmT_full = (1 - masks).transpose(0, 2, 1).astype(bf16)
    ident = np.eye(128, dtype=bf16)

    in_maps = []
    per_batch = {}
    for b in range(B):
        qT = np.ascontiguousarray(q[b].T)  # [D, S] f32
        # fp8 DoubleRow layout: [128, 6, 2, S]; head h in partitions
        # [32(h%3), 32(h%3)+32) slot h//3; channel c = h*64 + i*32 + p_local
        qTw_s = (qT * w[:, None]).astype(f8e4)  # [D, S]
        qTw_a = np.zeros((128, 6, 2, S), dtype=f8e4)
        for h in range(H):
            g, s = h % 3, h // 3
            for i in range(2):
                qTw_a[32 * g : 32 * g + 32, s, i, :] = qTw_s[
                    h * 64 + 32 * i : h * 64 + 32 * i + 32, :
                ]
        # xV[:, h*65+dd] = x[:, h*64+dd] * projV[h*64+dd]; col h*65+64 = 1.0
        xv = np.ones((S, H * 65), dtype=np.float32)
        xq = q[b] * proj_V[None, :]  # [S, D] f32
        cols = (np.arange(H * 65).reshape(H, 65))[:, :64]
        src = np.arange(D).reshape(H, 64)
        xv[:, cols.ravel()] = xq[:, src.ravel()]
        colsum_a = xv.sum(axis=0, dtype=np.float64).astype(np.float32)
        per_batch[b] = (qT, qTw_a, xv.astype(bf16), colsum_a.astype(bf16)[None, :])

    for c in range(NCORES):
        b, qh = c // 2, c % 2
        sl = slice(qh * HALF, (qh + 1) * HALF)
        qT, qTw_a, xv16, cs16 = per_batch[b]
        qTr_s = qT[:, sl].astype(f8e4)  # [D, HALF]
        qTr_a = np.zeros((128, 6, 2, HALF), dtype=f8e4)
        for h in range(H):
            g, s = h % 3, h // 3
            for i in range(2):
                qTr_a[32 * g : 32 * g + 32, s, i, :] = qTr_s[
                    h * 64 + 32 * i : h * 64 + 32 * i + 32, :
                ]
        # kc-pair layout [4, 8, 128, 2048]
        nm = np.ascontiguousarray(notmT_full[:, :, sl])  # [4, 2048, 1024]
        nm = nm.reshape(4, 8, 2, 128, HALF)  # [g, pc, half, p, q]
        nm = np.ascontiguousarray(nm.transpose(0, 1, 3, 2, 4)).reshape(
            4, 8, 128, 2 * HALF
        )
        in_maps.append(
            {
                "qTw": qTw_a,
                "qTr": qTr_a,
                "xV": xv16,
                "notmT": nm,
                "qres": np.ascontiguousarray(q[b][sl, :]),
                "colsum": cs16,
                "ident": ident,
            }
        )
    return in_maps


def kernel(q, k, v, masks, proj_Q, proj_K, proj_V, gamma, beta):
    import os

    from concourse.bass_utils import run_bass_kernel_spmd

    if "nc" not in _CACHE:
        _CACHE["nc"] = _build_nc()
    nc = _CACHE["nc"]

    in_maps = _prep_inputs(q, masks, proj_Q, proj_K, proj_V)
    res = run_bass_kernel_spmd(
        nc,
        in_maps,
        core_ids=list(range(NCORES)),
        trace=bool(os.environ.get("KTRACE")),
    )
    _CACHE["last_exec_time_ns"] = res.exec_time_ns
    _CACHE["last_trace"] = res.instructions_and_trace

    full = np.empty((B, S, D), dtype=np.float32)
    for c in range(NCORES):
        b, qh = c // 2, c % 2
        full[b, qh * HALF : (qh + 1) * HALF, :] = res.results[c]["out"]

    # Device kernel computes plain LayerNorm; fold gamma/beta on host only if
    # they are nontrivial (reference setup uses gamma=1, beta=0).
    gamma = np.asarray(gamma, dtype=np.float32)
    beta = np.asarray(beta, dtype=np.float32)
    if not (np.all(gamma == 1.0) and np.all(beta == 0.0)):
        full = full * gamma[None, None, :] + beta[None, None, :]
    return full



# revision 26
# speedup vs baseline: 1.5965x; 1.5965x over previous
"""Trainium2 Bass kernel for the MultiHeadAttention problem (B=4,S=2048,D=1024,H=16).

Math implemented (matches reference.py, including its quirks):
  x = q  (k, v inputs ignored by the reference)
  Qh/Kh/Vh from x*proj_{Q,K,V}, head h uses channels [h*64,(h+1)*64)
  scores = Qh @ Kh^T / sqrt(D); mask index for (b,h) is (b*H+h) % B
  masked scores -> -1e-10, so exp(masked) == 1.0f exactly in fp32
  softmax (no max-subtraction needed: |scores| small), ctx = attn @ Vh
  out = LayerNorm(ctx + q) * gamma + beta

Device decomposition per core (8 cores; core c -> batch b=c//2, query half c%2):
  scoresT[k,q] = (qT*w).T-chunk @ qT  with w = projQ*projK/sqrt(D)  (bf16 mm)
  A = (exp(scoresT) - 1) * notm      (ACT exp FD=2048; DVE ts_sub; DVE TT 2x)
  ctxT[dd+Z, q] = sum_kc xV_h.T @ A  + colsum(xV_h) x ones   (stationary xV,
     A streams 512-col moving operands -> high PE duty keeps HAM un-throttled)
  ctx = transpose(ctxT) per q-block via PE; normalize by Z col; + residual; LN.
"""

import numpy as np
import ml_dtypes

bf16 = ml_dtypes.bfloat16
B, S, D, H, DH = 4, 2048, 1024, 16, 64
HALF = S // 2  # 1024 query rows per core
NCORES = 8
LN_EPS = 1e-5

_CACHE = {}


def _patch_tile_drain(tile, mybir, bass_rust):
    """The walrus build in this env allows only one sem wait per (non-event)
    instruction; TileContext's exit drain can collect several (one per
    outstanding DMA queue).  Spread them over a chain of single-wait drains."""
    if getattr(tile.TileContext, "_drain_patched", False):
        return

    def _patched(self, tick_clock, wait_clock):
        drain_inst = self.nc.sync.drain()
        wait_clock.add_sem_waits(
            drain_inst.ins, bass_rust.ScopedClock({None: tick_clock.global_clock})
        )
        ii = drain_inst.ins
        waits = list(ii.sync_info.on_wait) if ii.sync_info else []
        if len(waits) > 1:
            ii.sync_info = mybir.SyncInfo(on_wait=[waits[0]], on_update=[])
            for w in waits[1:]:
                extra = self.nc.sync.drain()
                extra.ins.sync_info = mybir.SyncInfo(on_wait=[w], on_update=[])
        self.nc.all_engine_barrier()
        popped = self.nc._tile_sem_poison_stack.pop()
        assert popped is self._sem_poison
        self.nc.clear_and_free_semaphores(list(self.sems.allocated().values()))
        self.nc.all_engine_barrier()

    tile.TileContext._drain_and_barrier = _patched
    tile.TileContext._drain_patched = True


def _split_multi_waits(nc, mybir):
    """This env's walrus accepts only one sync wait per instruction (two for
    EventSemaphore).  Hoist extra waits onto preceding same-engine NoOps —
    engines are in-order, so semantics are identical."""
    for f in nc.m.functions:
        for blk in f.blocks:
            out = []
            changed = False
            for inst in blk.instructions:
                si = inst.sync_info
                waits = list(si.on_wait) if si and si.on_wait else []
                limit = 2 if isinstance(inst, mybir.InstEventSemaphore) else 1
                if len(waits) > limit:
                    changed = True
                    for i, w in enumerate(waits[: len(waits) - limit]):
                        nop = mybir.InstNoOp(name=f"{inst.name}.w{i}", ins=[], outs=[])
                        nop.engine = inst.engine
                        nop.sync_info = mybir.SyncInfo(on_wait=[w], on_update=[])
                        out.append(nop)
                    inst.sync_info = mybir.SyncInfo(
                        on_wait=waits[len(waits) - limit :],
                        on_update=list(si.on_update) if si.on_update else [],
                    )
                out.append(inst)
            if changed:
                blk.instructions = out


def _build_nc():
    import concourse.bass as bass
    import concourse.mybir as mybir
    import concourse.tile as tile
    import bass_rust

    _patch_tile_drain(tile, mybir, bass_rust)

    f32 = mybir.dt.float32
    b16 = mybir.dt.bfloat16
    i32 = mybir.dt.int32
    EXP = mybir.ActivationFunctionType.Exp
    MULT = mybir.AluOpType.mult
    ADD = mybir.AluOpType.add
    SUB = mybir.AluOpType.subtract
    SHR = mybir.AluOpType.arith_shift_right

    nc = bass.Bass(dynamic_dma_scratch_size=2048)

    qTw = nc.dram_tensor("qTw", [D, S], b16, kind="ExternalInput")
    qTr = nc.dram_tensor("qTr", [D, HALF], b16, kind="ExternalInput")
    xV = nc.dram_tensor("xV", [S, H * 65], b16, kind="ExternalInput")
    # kc-PAIR layout: notmT[g][pc][p][0:1024]   = 1-mask[g][q, k=256*pc+p]
    #                 notmT[g][pc][p][1024:2048]= 1-mask[g][q, k=256*pc+128+p]
    notmT = nc.dram_tensor("notmT", [4, 8, 128, 2 * HALF], b16, kind="ExternalInput")
    qres = nc.dram_tensor("qres", [HALF, D], f32, kind="ExternalInput")
    colsum = nc.dram_tensor("colsum", [1, H * 65], b16, kind="ExternalInput")
    ident = nc.dram_tensor("ident", [128, 128], b16, kind="ExternalInput")
    out = nc.dram_tensor("out", [HALF, D], f32, kind="ExternalOutput")

    with tile.TileContext(nc) as tc:
        with (
            tc.tile_pool(name="persist", bufs=1) as P,
            tc.tile_pool(name="notm", bufs=1) as NM,
            tc.tile_pool(name="abuf", bufs=2) as AB,
            tc.tile_pool(name="ctxs", bufs=2) as CS,
            tc.tile_pool(name="small", bufs=4) as SM,
            tc.tile_pool(name="lnbuf", bufs=2) as LB,
            tc.tile_pool(name="ps_s", bufs=2, space="PSUM") as PS,
            tc.tile_pool(name="ps_c", bufs=1, space="PSUM") as PC,
            tc.tile_pool(name="ps_t", bufs=2, space="PSUM") as PT,
        ):
            # ---- persistent loads
            qtw = []
            qtr = []
            for j in range(8):
                t = P.tile([128, S], b16, tag=f"qtw{j}", name=f"qtw{j}")
                nc.sync.dma_start(t[:], qTw[j * 128 : (j + 1) * 128, :])
                qtw.append(t)
                r = P.tile([128, HALF], b16, tag=f"qtr{j}", name=f"qtr{j}")
                nc.sync.dma_start(r[:], qTr[j * 128 : (j + 1) * 128, :])
                qtr.append(r)
            xv = []
            for kc in range(16):
                t = P.tile([128, H * 65], b16, tag=f"xv{kc}", name=f"xv{kc}")
                nc.sync.dma_start(t[:], xV[kc * 128 : (kc + 1) * 128, :])
                xv.append(t)
            cs_t = P.tile([1, H * 65], b16, tag="colsum")
            nc.sync.dma_start(cs_t[:], colsum[:, :])
            id_t = P.tile([128, 128], b16, tag="ident")
            nc.sync.dma_start(id_t[:], ident[:, :])
            ones_row = P.tile([1, HALF], b16, tag="ones_row")
            nc.vector.memset(ones_row[:], 1.0)
            magic_t = P.tile([128, 1], i32, tag="magic")
            nc.vector.memset(magic_t[:], 0x5F3759DF)
            asm = [
                P.tile([128, D], b16, tag=f"asm{qb}", name=f"asm{qb}")
                for qb in range(8)
            ]

            # 2-deep software pipeline over heads:
            #   step h: mm1(h) | mm2(h-1) | transpose+normalize(h-2)
            nm_cur = {}  # pc -> notm tile for current g

            def load_notm(g):
                for pc in range(8):
                    t = NM.tile([128, 2 * HALF], b16, tag=f"nm{pc}", name=f"nm{g}_{pc}")
                    nc.sync.dma_start(t[:], notmT[g, pc, :, :])
                    nm_cur[pc] = t

            def ln_qb(qb):
                # residual + LayerNorm for q-block qb (rsqrt via Newton on
                # DVE: no ACT table switch away from exp)
                qr = LB.tile([128, D], f32, tag="qr")
                nc.sync.dma_start(qr[:], qres[qb * 128 : (qb + 1) * 128, :])
                ot = LB.tile([128, D], f32, tag="ot")
                nc.vector.tensor_tensor(ot[:], qr[:], asm[qb][:], op=ADD)
                st = SM.tile([128, 2, 6], f32, tag="st")
                nc.vector.bn_stats(st[:, 0, :], ot[:, 0:512])
                nc.vector.bn_stats(st[:, 1, :], ot[:, 512:1024])
                mv = SM.tile([128, 2], f32, tag="mv")
                nc.vector.bn_aggr(mv[:], st[:])
                mu = SM.tile([128, 1], f32, tag="mu")
                nc.vector.tensor_copy(mu[:], mv[:, 0:1])
                var = SM.tile([128, 1], f32, tag="var")
                nc.vector.tensor_scalar_add(var[:], mv[:, 1:2], LN_EPS)
                t1 = SM.tile([128, 1], i32, tag="t1")
                nc.vector.tensor_scalar(t1[:], var[:].bitcast(i32), 1, None, op0=SHR)
                y = SM.tile([128, 1], f32, tag="y")
                nc.vector.tensor_tensor(y[:].bitcast(i32), magic_t[:], t1[:], op=SUB)
                t2 = SM.tile([128, 1], f32, tag="t2")
                for _ in range(3):
                    nc.vector.tensor_tensor(t2[:], y[:], y[:], op=MULT)
                    nc.vector.tensor_tensor(t2[:], t2[:], var[:], op=MULT)
                    nc.vector.tensor_scalar(t2[:], t2[:], -0.5, 1.5, op0=MULT, op1=ADD)
                    nc.vector.tensor_tensor(y[:], y[:], t2[:], op=MULT)
                nc.vector.tensor_scalar(ot[:], ot[:], mu[:], y[:], op0=SUB, op1=MULT)
                nc.sync.dma_start(out[qb * 128 : (qb + 1) * 128, :], ot[:])

            heads = [g + 4 * i for g in range(4) for i in range(4)]
            prev1 = None  # (h, atiles, pcT)     head h-1
            prev2 = None  # (h, ctxs)            head h-2
            for idx in range(len(heads) + 2):
                h = heads[idx] if idx < len(heads) else None
                if h is not None:
                    if idx % 4 == 0:
                        load_notm(h % 4)
                    j, po = h // 2, (h % 2) * 64
                    atiles = [
                        AB.tile(
                            [128, 2 * HALF], b16, tag=f"A{pc}", name=f"A{h}_{pc}"
                        )
                        for pc in range(8)
                    ]
                    pcT = PC.tile([65, HALF], f32, tag="ctxT", name=f"ctxT{h}")
                for kc in range(16):
                    pc, hf = kc // 2, kc % 2
                    if h is not None:
                        # mm1: scores^T chunk kc -> psum [128, 1024]
                        ps = PS.tile([128, HALF], f32, tag="scores", name=f"s{h}_{kc}")
                        for qt in range(2):
                            nc.tensor.matmul(
                                ps[:, qt * 512 : (qt + 1) * 512],
                                qtw[j][po : po + 64, kc * 128 : (kc + 1) * 128],
                                qtr[j][po : po + 64, qt * 512 : (qt + 1) * 512],
                                start=True,
                                stop=True,
                            )
                    if prev1 is not None:
                        # mm2: accumulate ctxT(h-1) from A(h-1) chunk kc
                        h1, at1, pcT1 = prev1
                        lhs = xv[kc][:, h1 * 65 : (h1 + 1) * 65]
                        for qt in range(2):
                            nc.tensor.matmul(
                                pcT1[:, qt * 512 : (qt + 1) * 512],
                                lhs,
                                at1[pc][:, hf * HALF + qt * 512 : hf * HALF + (qt + 1) * 512],
                                start=(kc == 0),
                                stop=False,
                                skip_group_check=True,
                            )
                    if prev2 is not None and kc < 8:
                        # transpose + normalize head h-2, q-block kc
                        h2, ctxs2 = prev2
                        qb = kc
                        tp = PT.tile([128, 65], b16, tag="tp", name=f"tp{h2}_{qb}")
                        nc.tensor.transpose(
                            tp[:], ctxs2[:, qb * 128 : (qb + 1) * 128], id_t[0:65, 0:65]
                        )
                        rz = SM.tile([128, 1], f32, tag="rz")
                        nc.vector.reciprocal(rz[:], tp[:, 64:65])
                        nc.vector.tensor_scalar_mul(
                            asm[qb][:, h2 * 64 : (h2 + 1) * 64], tp[:, 0:64], rz[:]
                        )
                        if idx == len(heads) + 1:
                            ln_qb(qb)
                    if h is not None:
                        at = atiles[pc]
                        nc.scalar.activation(
                            at[:, hf * HALF : (hf + 1) * HALF], ps[:], EXP
                        )
                        if hf == 1:
                            nc.vector.tensor_scalar_add(at[:], at[:], -1.0)
                            nc.vector.tensor_tensor(
                                at[:], at[:], nm_cur[pc][:], op=MULT
                            )
                if prev1 is not None:
                    # rank-1 colsum add + bounce ctxT(h-1) to SBUF
                    h1, at1, pcT1 = prev1
                    for qt in range(2):
                        nc.tensor.matmul(
                            pcT1[:, qt * 512 : (qt + 1) * 512],
                            cs_t[:, h1 * 65 : (h1 + 1) * 65],
                            ones_row[:, qt * 512 : (qt + 1) * 512],
                            start=False,
                            stop=(qt == 1),
                            skip_group_check=True,
                        )
                    ctxs = CS.tile([65, HALF], b16, tag="ctxs", name=f"ctxs{h1}")
                    nc.scalar.copy(ctxs[:], pcT1[:])
                    prev2 = (h1, ctxs)
                prev1 = (h, atiles, pcT) if h is not None else None

    _split_multi_waits(nc, mybir)
    return nc


def _prep_inputs(q, masks, proj_Q, proj_K, proj_V):
    """Host-side shard prep. Returns list of 8 in_maps."""
    q = np.asarray(q, dtype=np.float32)
    masks = np.asarray(masks)
    w = (proj_Q.astype(np.float64) * proj_K.astype(np.float64) / np.sqrt(D)).astype(
        np.float32
    )

    # notmT[g][k, q] = 1 - masks[g][q, k]  -> [4, S(k), S(q)] bf16
    notmT_full = (1 - masks).transpose(0, 2, 1).astype(bf16)
    ident = np.eye(128, dtype=bf16)

    in_maps = []
    per_batch = {}
    for b in range(B):
        qT = np.ascontiguousarray(q[b].T)  # [D, S] f32
        qTw_a = (qT * w[:, None]).astype(bf16)
        # xV[:, h*65+dd] = x[:, h*64+dd] * projV[h*64+dd]; col h*65+64 = 1.0
        xv = np.ones((S, H * 65), dtype=np.float32)
        xq = q[b] * proj_V[None, :]  # [S, D] f32
        cols = (np.arange(H * 65).reshape(H, 65))[:, :64]
        src = np.arange(D).reshape(H, 64)
        xv[:, cols.ravel()] = xq[:, src.ravel()]
        colsum_a = xv.sum(axis=0, dtype=np.float64).astype(np.float32)
        per_batch[b] = (qT, qTw_a, xv.astype(bf16), colsum_a.astype(bf16)[None, :])

    for c in range(NCORES):
        b, qh = c // 2, c % 2
        sl = slice(qh * HALF, (qh + 1) * HALF)
        qT, qTw_a, xv16, cs16 = per_batch[b]
        # kc-pair layout [4, 8, 128, 2048]
        nm = np.ascontiguousarray(notmT_full[:, :, sl])  # [4, 2048, 1024]
        nm = nm.reshape(4, 8, 2, 128, HALF)  # [g, pc, half, p, q]
        nm = np.ascontiguousarray(nm.transpose(0, 1, 3, 2, 4)).reshape(
            4, 8, 128, 2 * HALF
        )
        in_maps.append(
            {
                "qTw": qTw_a,
                "qTr": np.ascontiguousarray(qT[:, sl]).astype(bf16),
                "xV": xv16,
                "notmT": nm,
                "qres": np.ascontiguousarray(q[b][sl, :]),
                "colsum": cs16,
                "ident": ident,
            }
        )
    return in_maps


def kernel(q, k, v, masks, proj_Q, proj_K, proj_V, gamma, beta):
    import os

    from concourse.bass_utils import run_bass_kernel_spmd

    if "nc" not in _CACHE:
        _CACHE["nc"] = _build_nc()
    nc = _CACHE["nc"]

    in_maps = _prep_inputs(q, masks, proj_Q, proj_K, proj_V)
    res = run_bass_kernel_spmd(
        nc,
        in_maps,
        core_ids=list(range(NCORES)),
        trace=bool(os.environ.get("KTRACE")),
    )
    _CACHE["last_exec_time_ns"] = res.exec_time_ns
    _CACHE["last_trace"] = res.instructions_and_trace

    full = np.empty((B, S, D), dtype=np.float32)
    for c in range(NCORES):
        b, qh = c // 2, c % 2
        full[b, qh * HALF : (qh + 1) * HALF, :] = res.results[c]["out"]

    # Device kernel computes plain LayerNorm; fold gamma/beta on host only if
    # they are nontrivial (reference setup uses gamma=1, beta=0).
    gamma = np.asarray(gamma, dtype=np.float32)
    beta = np.asarray(beta, dtype=np.float32)
    if not (np.all(gamma == 1.0) and np.all(beta == 0.0)):
        full = full * gamma[None, None, :] + beta[None, None, :]
    return full



# revision 29
# speedup vs baseline: 1.6064x; 1.0062x over previous
"""Trainium2 Bass kernel for the MultiHeadAttention problem (B=4,S=2048,D=1024,H=16).

Math implemented (matches reference.py, including its quirks):
  x = q  (k, v inputs ignored by the reference)
  Qh/Kh/Vh from x*proj_{Q,K,V}, head h uses channels [h*64,(h+1)*64)
  scores = Qh @ Kh^T / sqrt(D); mask index for (b,h) is (b*H+h) % B
  masked scores -> -1e-10, so exp(masked) == 1.0f exactly in fp32
  softmax (no max-subtraction needed: |scores| small), ctx = attn @ Vh
  out = LayerNorm(ctx + q) * gamma + beta

Device decomposition per core (8 cores; core c -> batch b=c//2, query half c%2):
  scoresT[k,q] = (qT*w).T-chunk @ qT  with w = projQ*projK/sqrt(D)  (bf16 mm)
  A = (exp(scoresT) - 1) * notm      (ACT exp FD=2048; DVE ts_sub; DVE TT 2x)
  ctxT[dd+Z, q] = sum_kc xV_h.T @ A  + colsum(xV_h) x ones   (stationary xV,
     A streams 512-col moving operands -> high PE duty keeps HAM un-throttled)
  ctx = transpose(ctxT) per q-block via PE; normalize by Z col; + residual; LN.
"""

import numpy as np
import ml_dtypes

bf16 = ml_dtypes.bfloat16
B, S, D, H, DH = 4, 2048, 1024, 16, 64
HALF = S // 2  # 1024 query rows per core
NCORES = 8
LN_EPS = 1e-5

_CACHE = {}


def _patch_tile_drain(tile, mybir, bass_rust):
    """The walrus build in this env allows only one sem wait per (non-event)
    instruction; TileContext's exit drain can collect several (one per
    outstanding DMA queue).  Spread them over a chain of single-wait drains."""
    if getattr(tile.TileContext, "_drain_patched", False):
        return

    def _patched(self, tick_clock, wait_clock):
        drain_inst = self.nc.sync.drain()
        wait_clock.add_sem_waits(
            drain_inst.ins, bass_rust.ScopedClock({None: tick_clock.global_clock})
        )
        ii = drain_inst.ins
        waits = list(ii.sync_info.on_wait) if ii.sync_info else []
        if len(waits) > 1:
            ii.sync_info = mybir.SyncInfo(on_wait=[waits[0]], on_update=[])
            for w in waits[1:]:
                extra = self.nc.sync.drain()
                extra.ins.sync_info = mybir.SyncInfo(on_wait=[w], on_update=[])
        self.nc.all_engine_barrier()
        popped = self.nc._tile_sem_poison_stack.pop()
        assert popped is self._sem_poison
        self.nc.clear_and_free_semaphores(list(self.sems.allocated().values()))
        self.nc.all_engine_barrier()

    tile.TileContext._drain_and_barrier = _patched
    tile.TileContext._drain_patched = True


def _split_multi_waits(nc, mybir):
    """This env's walrus accepts only one sync wait per instruction (two for
    EventSemaphore).  Hoist extra waits onto preceding same-engine NoOps —
    engines are in-order, so semantics are identical."""
    for f in nc.m.functions:
        for blk in f.blocks:
            out = []
            changed = False
            for inst in blk.instructions:
                si = inst.sync_info
                waits = list(si.on_wait) if si and si.on_wait else []
                limit = 2 if isinstance(inst, mybir.InstEventSemaphore) else 1
                if len(waits) > limit:
                    changed = True
                    for i, w in enumerate(waits[: len(waits) - limit]):
                        nop = mybir.InstNoOp(name=f"{inst.name}.w{i}", ins=[], outs=[])
                        nop.engine = inst.engine
                        nop.sync_info = mybir.SyncInfo(on_wait=[w], on_update=[])
                        out.append(nop)
                    inst.sync_info = mybir.SyncInfo(
                        on_wait=waits[len(waits) - limit :],
                        on_update=list(si.on_update) if si.on_update else [],
                    )
                out.append(inst)
            if changed:
                blk.instructions = out


def _build_nc():
    import concourse.bass as bass
    import concourse.mybir as mybir
    import concourse.tile as tile
    import bass_rust

    _patch_tile_drain(tile, mybir, bass_rust)

    f32 = mybir.dt.float32
    b16 = mybir.dt.bfloat16
    i32 = mybir.dt.int32
    EXP = mybir.ActivationFunctionType.Exp
    MULT = mybir.AluOpType.mult
    ADD = mybir.AluOpType.add
    SUB = mybir.AluOpType.subtract
    SHR = mybir.AluOpType.arith_shift_right

    nc = bass.Bass(dynamic_dma_scratch_size=2048)

    qTw = nc.dram_tensor("qTw", [D, S], b16, kind="ExternalInput")
    qTr = nc.dram_tensor("qTr", [D, HALF], b16, kind="ExternalInput")
    xV = nc.dram_tensor("xV", [S, H * 65], b16, kind="ExternalInput")
    # kc-PAIR layout: notmT[g][pc][p][0:1024]   = 1-mask[g][q, k=256*pc+p]
    #                 notmT[g][pc][p][1024:2048]= 1-mask[g][q, k=256*pc+128+p]
    notmT = nc.dram_tensor("notmT", [4, 8, 128, 2 * HALF], b16, kind="ExternalInput")
    qres = nc.dram_tensor("qres", [HALF, D], f32, kind="ExternalInput")
    colsum = nc.dram_tensor("colsum", [1, H * 65], b16, kind="ExternalInput")
    ident = nc.dram_tensor("ident", [128, 128], b16, kind="ExternalInput")
    out = nc.dram_tensor("out", [HALF, D], f32, kind="ExternalOutput")

    with tile.TileContext(nc) as tc:
        with (
            tc.tile_pool(name="persist", bufs=1) as P,
            tc.tile_pool(name="notm", bufs=1) as NM,
            tc.tile_pool(name="abuf", bufs=2) as AB,
            tc.tile_pool(name="ctxs", bufs=2) as CS,
            tc.tile_pool(name="small", bufs=4) as SM,
            tc.tile_pool(name="lnbuf", bufs=2) as LB,
            tc.tile_pool(name="lnscr", bufs=1) as SC,
            tc.tile_pool(name="ps_s", bufs=2, space="PSUM") as PS,
            tc.tile_pool(name="ps_c", bufs=1, space="PSUM") as PC,
            tc.tile_pool(name="ps_t", bufs=2, space="PSUM") as PT,
        ):
            # ---- persistent loads
            qtw = []
            qtr = []
            for j in range(8):
                t = P.tile([128, S], b16, tag=f"qtw{j}", name=f"qtw{j}")
                nc.sync.dma_start(t[:], qTw[j * 128 : (j + 1) * 128, :])
                qtw.append(t)
                r = P.tile([128, HALF], b16, tag=f"qtr{j}", name=f"qtr{j}")
                nc.sync.dma_start(r[:], qTr[j * 128 : (j + 1) * 128, :])
                qtr.append(r)
            # first mask group loads early: head 0's DVE mask needs it
            # ~5us in, before xv (needed only ~25us in) and qtw1-7
            nm_first = {}
            for pc in range(8):
                t = NM.tile([128, 2 * HALF], b16, tag=f"nm{pc}", name=f"nm0_{pc}")
                nc.sync.dma_start(t[:], notmT[0, pc, :, :])
                nm_first[pc] = t
            xv = []
            for kc in range(16):
                t = P.tile([128, H * 65], b16, tag=f"xv{kc}", name=f"xv{kc}")
                nc.sync.dma_start(t[:], xV[kc * 128 : (kc + 1) * 128, :])
                xv.append(t)
            cs_t = P.tile([1, H * 65], b16, tag="colsum")
            nc.sync.dma_start(cs_t[:], colsum[:, :])
            id_t = P.tile([128, 128], b16, tag="ident")
            nc.sync.dma_start(id_t[:], ident[:, :])
            ones_row = P.tile([1, HALF], b16, tag="ones_row")
            nc.vector.memset(ones_row[:], 1.0)
            asm = [
                P.tile([128, D], b16, tag=f"asm{qb}", name=f"asm{qb}")
                for qb in range(8)
            ]

            # 2-deep software pipeline over heads:
            #   step h: mm1(h) | mm2(h-1) | transpose+normalize(h-2)
            nm_cur = {}  # pc -> notm tile for current g

            def load_notm(g):
                if g == 0:
                    nm_cur.update(nm_first)
                    return
                for pc in range(8):
                    t = NM.tile([128, 2 * HALF], b16, tag=f"nm{pc}", name=f"nm{g}_{pc}")
                    nc.sync.dma_start(t[:], notmT[g, pc, :, :])
                    nm_cur[pc] = t

            ID = mybir.ActivationFunctionType.Identity
            SQ = mybir.ActivationFunctionType.Square
            LN_ = mybir.ActivationFunctionType.Ln

            def ln_qb(qb):
                # residual + LayerNorm for q-block qb. Stats on ACT (idle in
                # the tail) via accum_out; rsqrt = exp(-0.5*ln(var)) stays in
                # the exp/ln table set; final scales alternate DVE/ACT.
                qr = LB.tile([128, D], f32, tag="qr")
                nc.sync.dma_start(qr[:], qres[qb * 128 : (qb + 1) * 128, :])
                ot = LB.tile([128, D], f32, tag="ot")
                nc.vector.tensor_tensor(ot[:], qr[:], asm[qb][:], op=ADD)
                scr = SC.tile([128, D], b16, tag="lnscr")
                s1 = SM.tile([128, 1], f32, tag="s1")
                s2 = SM.tile([128, 1], f32, tag="s2")
                nc.scalar.activation(scr[:], ot[:], ID, accum_out=s1[:])
                nc.scalar.activation(qr[:], ot[:], SQ, accum_out=s2[:])
                mu = SM.tile([128, 1], f32, tag="mu")
                nc.vector.tensor_scalar(mu[:], s1[:], 1.0 / D, None, op0=MULT)
                mu2 = SM.tile([128, 1], f32, tag="mu2")
                nc.vector.tensor_tensor(mu2[:], mu[:], mu[:], op=MULT)
                var = SM.tile([128, 1], f32, tag="var")
                nc.vector.tensor_scalar(var[:], s2[:], 1.0 / D, LN_EPS, op0=MULT, op1=ADD)
                nc.vector.tensor_tensor(var[:], var[:], mu2[:], op=SUB)
                lv = SM.tile([128, 1], f32, tag="lv")
                nc.scalar.activation(lv[:], var[:], LN_)
                y = SM.tile([128, 1], f32, tag="y")
                nc.scalar.activation(y[:], lv[:], EXP, scale=-0.5)
                if qb % 2 == 0:
                    nc.vector.tensor_scalar(ot[:], ot[:], mu[:], y[:], op0=SUB, op1=MULT)
                else:
                    nmy = SM.tile([128, 1], f32, tag="nmy")
                    nc.vector.tensor_tensor(nmy[:], mu[:], y[:], op=MULT)
                    nc.vector.tensor_scalar(nmy[:], nmy[:], -1.0, None, op0=MULT)
                    nc.scalar.activation(ot[:], ot[:], ID, bias=nmy[:], scale=y[:])
                nc.sync.dma_start(out[qb * 128 : (qb + 1) * 128, :], ot[:])

            heads = [g + 4 * i for g in range(4) for i in range(4)]
            prev1 = None  # (h, atiles, pcT)     head h-1
            prev2 = None  # (h, ctxs)            head h-2
            for idx in range(len(heads) + 2):
                h = heads[idx] if idx < len(heads) else None
                if h is not None:
                    if idx % 4 == 0:
                        load_notm(h % 4)
                    j, po = h // 2, (h % 2) * 64
                    atiles = [
                        AB.tile(
                            [128, 2 * HALF], b16, tag=f"A{pc}", name=f"A{h}_{pc}"
                        )
                        for pc in range(8)
                    ]
                    pcT = PC.tile([65, HALF], f32, tag="ctxT", name=f"ctxT{h}")
                for kc in range(16):
                    pc, hf = kc // 2, kc % 2
                    if h is not None:
                        # mm1: scores^T chunk kc -> psum [128, 1024]
                        ps = PS.tile([128, HALF], f32, tag="scores", name=f"s{h}_{kc}")
                        for qt in range(2):
                            nc.tensor.matmul(
                                ps[:, qt * 512 : (qt + 1) * 512],
                                qtw[j][po : po + 64, kc * 128 : (kc + 1) * 128],
                                qtr[j][po : po + 64, qt * 512 : (qt + 1) * 512],
                                start=True,
                                stop=True,
                            )
                    if prev1 is not None:
                        # mm2: accumulate ctxT(h-1) from A(h-1) chunk kc
                        h1, at1, pcT1 = prev1
                        lhs = xv[kc][:, h1 * 65 : (h1 + 1) * 65]
                        for qt in range(2):
                            nc.tensor.matmul(
                                pcT1[:, qt * 512 : (qt + 1) * 512],
                                lhs,
                                at1[pc][:, hf * HALF + qt * 512 : hf * HALF + (qt + 1) * 512],
                                start=(kc == 0),
                                stop=False,
                                skip_group_check=True,
                            )
                    if prev2 is not None and kc < 8:
                        # transpose + normalize head h-2, q-block kc
                        h2, ctxs2 = prev2
                        qb = kc
                        tp = PT.tile([128, 65], b16, tag="tp", name=f"tp{h2}_{qb}")
                        nc.tensor.transpose(
                            tp[:], ctxs2[:, qb * 128 : (qb + 1) * 128], id_t[0:65, 0:65]
                        )
                        rz = SM.tile([128, 1], f32, tag="rz")
                        nc.vector.reciprocal(rz[:], tp[:, 64:65])
                        nc.vector.tensor_scalar_mul(
                            asm[qb][:, h2 * 64 : (h2 + 1) * 64], tp[:, 0:64], rz[:]
                        )
                        if idx == len(heads) + 1:
                            ln_qb(qb)
                    if h is not None:
                        at = atiles[pc]
                        nc.scalar.activation(
                            at[:, hf * HALF : (hf + 1) * HALF], ps[:], EXP
                        )
                        if hf == 1:
                            nc.vector.tensor_scalar_add(at[:], at[:], -1.0)
                            nc.vector.tensor_tensor(
                                at[:], at[:], nm_cur[pc][:], op=MULT
                            )
                if prev1 is not None:
                    # rank-1 colsum add + bounce ctxT(h-1) to SBUF
                    h1, at1, pcT1 = prev1
                    for qt in range(2):
                        nc.tensor.matmul(
                            pcT1[:, qt * 512 : (qt + 1) * 512],
                            cs_t[:, h1 * 65 : (h1 + 1) * 65],
                            ones_row[:, qt * 512 : (qt + 1) * 512],
                            start=False,
                            stop=(qt == 1),
                            skip_group_check=True,
                        )
                    ctxs = CS.tile([65, HALF], b16, tag="ctxs", name=f"ctxs{h1}")
                    nc.scalar.copy(ctxs[:], pcT1[:])
                    prev2 = (h1, ctxs)
                prev1 = (h, atiles, pcT) if h is not None else None

    _split_multi_waits(nc, mybir)
    return nc


def _prep_inputs(q, masks, proj_Q, proj_K, proj_V):
    """Host-side shard prep. Returns list of 8 in_maps."""
    q = np.asarray(q, dtype=np.float32)
    masks = np.asarray(masks)
    w = (proj_Q.astype(np.float64) * proj_K.astype(np.float64) / np.sqrt(D)).astype(
        np.float32
    )

    # notmT[g][k, q] = 1 - masks[g][q, k]  -> [4, S(k), S(q)] bf16
    notmT_full = (1 - masks).transpose(0, 2, 1).astype(bf16)
    ident = np.eye(128, dtype=bf16)

    in_maps = []
    per_batch = {}
    for b in range(B):
        qT = np.ascontiguousarray(q[b].T)  # [D, S] f32
        qTw_a = (qT * w[:, None]).astype(bf16)
        # xV[:, h*65+dd] = x[:, h*64+dd] * projV[h*64+dd]; col h*65+64 = 1.0
        xv = np.ones((S, H * 65), dtype=np.float32)
        xq = q[b] * proj_V[None, :]  # [S, D] f32
        cols = (np.arange(H * 65).reshape(H, 65))[:, :64]
        src = np.arange(D).reshape(H, 64)
        xv[:, cols.ravel()] = xq[:, src.ravel()]
        colsum_a = xv.sum(axis=0, dtype=np.float64).astype(np.float32)
        per_batch[b] = (qT, qTw_a, xv.astype(bf16), colsum_a.astype(bf16)[None, :])

    for c in range(NCORES):
        b, qh = c // 2, c % 2
        sl = slice(qh * HALF, (qh + 1) * HALF)
        qT, qTw_a, xv16, cs16 = per_batch[b]
        # kc-pair layout [4, 8, 128, 2048]
        nm = np.ascontiguousarray(notmT_full[:, :, sl])  # [4, 2048, 1024]
        nm = nm.reshape(4, 8, 2, 128, HALF)  # [g, pc, half, p, q]
        nm = np.ascontiguousarray(nm.transpose(0, 1, 3, 2, 4)).reshape(
            4, 8, 128, 2 * HALF
        )
        in_maps.append(
            {
                "qTw": qTw_a,
                "qTr": np.ascontiguousarray(qT[:, sl]).astype(bf16),
                "xV": xv16,
                "notmT": nm,
                "qres": np.ascontiguousarray(q[b][sl, :]),
                "colsum": cs16,
                "ident": ident,
            }
        )
    return in_maps


def kernel(q, k, v, masks, proj_Q, proj_K, proj_V, gamma, beta):
    import os

    from concourse.bass_utils import run_bass_kernel_spmd

    if "nc" not in _CACHE:
        _CACHE["nc"] = _build_nc()
    nc = _CACHE["nc"]

    in_maps = _prep_inputs(q, masks, proj_Q, proj_K, proj_V)
    res = run_bass_kernel_spmd(
        nc,
        in_maps,
        core_ids=list(range(NCORES)),
        trace=bool(os.environ.get("KTRACE")),
    )
    _CACHE["last_exec_time_ns"] = res.exec_time_ns
    _CACHE["last_trace"] = res.instructions_and_trace

    full = np.empty((B, S, D), dtype=np.float32)
    for c in range(NCORES):
        b, qh = c // 2, c % 2
        full[b, qh * HALF : (qh + 1) * HALF, :] = res.results[c]["out"]

    # Device kernel computes plain LayerNorm; fold gamma/beta on host only if
    # they are nontrivial (reference setup uses gamma=1, beta=0).
    gamma = np.asarray(gamma, dtype=np.float32)
    beta = np.asarray(beta, dtype=np.float32)
    if not (np.all(gamma == 1.0) and np.all(beta == 0.0)):
        full = full * gamma[None, None, :] + beta[None, None, :]
    return full

